# revision 14
# baseline (speedup 1.0000x reference)
"""ColBERT maxsim scoring kernel for Trainium2 (8 NeuronCores, SPMD).

Problem: Q [128, 32, 128] f32, D [1024, 220, 128] f32, D_mask [1024, 220] i32,
nway=8.  out[b] = sum_q max_k where(mask[b,k], D[b] @ Q[b//8].T, -9999)[k, q]
for b in 0..1024.

Sharding: data-parallel over docs. Core c handles docs [128c, 128c+128) and
the matching 16 query batches.

Host-side prep (per core):
  - Padded doc-token rows of D are replaced by a copy of the doc's first
    real token row: duplicates never change the per-doc max, so no mask
    bias is needed on-device at all.  (Fully-padded docs -- impossible for
    this input distribution -- are patched on the host afterwards.)
  - D is cast to fp8 e3m4 (randn fits comfortably in its +-15.5 range;
    measured end-to-end rel err 2.9e-3 vs the 2e-2 gate) and pre-transposed
    to [DIM, 28160 doc-rows] so the device DMA is a plain contiguous
    stream; no on-chip transposes.  fp8 halves HBM traffic vs bf16.
  - Q is pre-transposed to [DIM, 512] bf16 (16 groups x 32 queries).

Per-core device program (raw bass engine streams, manual semaphores -- the
Tile framework's auto-semaphore teardown alone cost ~7us):
  - 8 chunk DMAs (2 query groups each, alternating sync/scalar HWDGE
    queues) stream D^T into SBUF, issued as each queue's first
    instructions.
  - Tensor stream: per group g, 4 col-tiled matmuls (tile_position=
    (0,32j)) put scores for doc pair (8g+2j, 8g+2j+1) in psum bank (g%8)
    partitions [32j, 32j+32); the last matmul bumps s_mm.
  - Vector stream: per group, one 3D reduce_max ([128, 2, 220] ->
    [128, 2]) into Mx [128, 32]; bumps s_red (also recycles the bank).
  - Sync stream DMAs Mx halves out after groups 7 and 15; the host sums
    the four 32-query partition blocks and de-interleaves docs.
"""

import numpy as np
import ml_dtypes

import concourse.bacc as bacc
import concourse.mybir as mybir
from concourse import bass_utils

F32 = mybir.dt.float32
BF16 = mybir.dt.bfloat16
FP8 = mybir.dt.float8e3

N_CORES = 8
B = 128          # query batches
QLEN = 32
DIM = 128
NWAY = 8
DLEN = 220
DOCS_PER_CORE = (B * NWAY) // N_CORES          # 128
ROWS_PER_CORE = DOCS_PER_CORE * DLEN           # 28160
GROUPS_PER_CORE = DOCS_PER_CORE // NWAY        # 16
GROW = NWAY * DLEN                             # 1760 rows per group
GCOLS = 2 * DLEN                               # 440 score cols per psum bank
CH_SIZES = [3, 3, 3, 3, 1, 1, 1, 1]            # groups per chunk (big->small)
CH_G0 = [sum(CH_SIZES[:i]) for i in range(len(CH_SIZES))]
N_CHUNK = len(CH_SIZES)
BANK = 512                                     # psum bank stride (f32 elems)
BIG = 9999.0

_CACHE = {}


def _build_module():
    """Trace + compile the per-core bass module (same program on all cores)."""
    if "nc" in _CACHE:
        return _CACHE["nc"]

    nc = bacc.Bacc("TRN2", target_bir_lowering=False, debug=False)

    dt_dram = nc.dram_tensor("dt_in", [DIM, ROWS_PER_CORE], FP8,
                             kind="ExternalInput")
    qt_dram = nc.dram_tensor("qt_in", [DIM, GROUPS_PER_CORE * QLEN], BF16,
                             kind="ExternalInput")
    out_dram = nc.dram_tensor("outp", [128, 32], F32, kind="ExternalOutput")

    from contextlib import ExitStack
    with ExitStack() as stack, (
        nc.semaphore("q_dma")) as q_dma:
      ch_sems = [stack.enter_context(nc.semaphore(f"c_dma{i}"))
                 for i in range(N_CHUNK)]
      with (
        nc.semaphore("s_mm") as s_mm,      # groups of matmuls retired
        nc.semaphore("s_red") as s_red,    # reduces retired (bank recycle)
        nc.semaphore("s_out") as s_out,    # output DMA completions
        nc.sbuf_tensor("qt_sb", [128, GROUPS_PER_CORE * QLEN], BF16) as qt_sb,
        nc.sbuf_tensor("dt_sb", [128, ROWS_PER_CORE], FP8) as dt_sb,
        nc.sbuf_tensor("mx_sb", [128, 32], F32) as mx_sb,
        nc.psum_tensor("ps", [128, 8 * BANK], F32) as ps,
        nc.Block(no_gpsimd_drain=True) as block,
      ):
        @block.sync
        def _(sync):
            # warmup: tiny transfer wakes the DMA engines early
            sync.dma_start(qt_sb[0:1, :], qt_dram.ap()[0:1, :]).then_inc(
                s_out, 16)
            for ch in range(0, N_CHUNK, 2):
                a, b = CH_G0[ch] * GROW, (CH_G0[ch] + CH_SIZES[ch]) * GROW
                sync.dma_start(
                    dt_sb[:, a:b], dt_dram.ap()[:, a:b],
                ).then_inc(ch_sems[ch], 16)
            for part in range(4):
                sync.wait_ge(s_red, 4 * (part + 1))
                sync.dma_start(out_dram.ap()[:, 8 * part:8 * (part + 1)],
                               mx_sb[:, 8 * part:8 * (part + 1)]
                               ).then_inc(s_out, 16)
            sync.wait_ge(s_out, 16 * 5)

        @block.scalar
        def _(scalar):
            scalar.dma_start(qt_sb[:, :], qt_dram.ap()).then_inc(q_dma, 16)
            for ch in range(1, N_CHUNK, 2):
                a, b = CH_G0[ch] * GROW, (CH_G0[ch] + CH_SIZES[ch]) * GROW
                scalar.dma_start(
                    dt_sb[:, a:b], dt_dram.ap()[:, a:b],
                ).then_inc(ch_sems[ch], 16)

        @block.tensor
        def _(tensor):
            tensor.wait_ge(q_dma, 16)
            g2ch = {}
            for ch, g0 in enumerate(CH_G0):
                for k in range(CH_SIZES[ch]):
                    g2ch[g0 + k] = ch
            for g in range(GROUPS_PER_CORE):
                ch = g2ch[g]
                if g == CH_G0[ch]:
                    tensor.wait_ge(ch_sems[ch], 16)
                if g >= 8:
                    tensor.wait_ge(s_red, g - 7)   # psum bank recycle
                b0 = (g % 8) * BANK
                c0 = g * GROW
                for j in range(4):
                    mm = tensor.matmul(
                        ps[32 * j:32 * (j + 1), b0:b0 + GCOLS],
                        lhsT=qt_sb[:, QLEN * g:QLEN * (g + 1)],
                        rhs=dt_sb[:, c0 + GCOLS * j:c0 + GCOLS * (j + 1)],
                        start=True, stop=True,
                        tile_position=(0, 32 * j),
                        skip_group_check=True,
                    )
                mm.then_inc(s_mm, 1)

        @block.vector
        def _(vector):
            for g in range(GROUPS_PER_CORE):
                vector.wait_ge(s_mm, g + 1)
                b0 = (g % 8) * BANK
                vector.tensor_reduce(
                    mx_sb[:, 2 * g:2 * g + 2],
                    ps[:, b0:b0 + GCOLS].rearrange("p (t k) -> p t k", t=2),
                    axis=mybir.AxisListType.X,
                    op=mybir.AluOpType.max,
                ).then_inc(s_red, 1)

    nc.compile()
    _CACHE["nc"] = nc
    return nc


def _in_maps(Q, D, D_mask):
    """Host-side prep: per-core input dicts (pad-fill + cast + transpose)."""
    mask = D_mask > 0
    first_real = np.argmax(mask, axis=1)                  # [1024]
    kk = np.arange(DLEN)[None, :]
    idx = np.where(mask, kk, first_real[:, None])         # [1024, 220]
    d_filled = np.take_along_axis(D, idx[:, :, None], axis=1)
    dt_all = np.ascontiguousarray(
        d_filled.reshape(N_CORES, ROWS_PER_CORE, DIM).transpose(0, 2, 1)
    ).astype(ml_dtypes.float8_e3m4)
    qt_all = np.ascontiguousarray(
        Q.reshape(N_CORES, GROUPS_PER_CORE * QLEN, DIM).transpose(0, 2, 1)
    ).astype(ml_dtypes.bfloat16)
    return [{"dt_in": dt_all[c], "qt_in": qt_all[c]} for c in range(N_CORES)]


def kernel(Q, D, D_mask, nway):
    assert int(nway) == NWAY
    Q = np.ascontiguousarray(np.asarray(Q, dtype=np.float32))
    D = np.ascontiguousarray(np.asarray(D, dtype=np.float32))
    D_mask = np.asarray(D_mask, dtype=np.int32)

    nc = _build_module()
    res = bass_utils.run_bass_kernel_spmd(nc, _in_maps(Q, D, D_mask),
                                          core_ids=list(range(N_CORES)))

    # outp[32j+q, 2g+t] = maxsim for doc (8g+2j+t), query q; sum over q.
    s = np.arange(32)
    j = np.arange(4)
    doc_idx = 8 * (s[None, :] // 2) + 2 * j[:, None] + (s[None, :] % 2)
    out = np.empty(B * NWAY, np.float32)
    for c in range(N_CORES):
        blk = res.results[c]["outp"].reshape(4, 32, 32).sum(axis=1)  # [j, s]
        per_core = np.empty(DOCS_PER_CORE, np.float32)
        per_core[doc_idx.ravel()] = blk.ravel()
        out[c * DOCS_PER_CORE:(c + 1) * DOCS_PER_CORE] = per_core

    # fully-padded docs: reference yields exactly 32 * -9999
    fully = ~(D_mask > 0).any(axis=1)
    if fully.any():
        out[fully] = np.float32(32 * -BIG)
    return out


# revision 15
# speedup vs baseline: 1.0569x; 1.0569x over previous
"""ColBERT maxsim scoring kernel for Trainium2 (8 NeuronCores, SPMD).

Problem: Q [128, 32, 128] f32, D [1024, 220, 128] f32, D_mask [1024, 220] i32,
nway=8.  out[b] = sum_q max_k where(mask[b,k], D[b] @ Q[b//8].T, -9999)[k, q]
for b in 0..1024.

Sharding: data-parallel over docs. Core c handles docs [128c, 128c+128) and
the matching 16 query batches.

Host-side prep (per core):
  - Padded doc-token rows of D are replaced by a copy of the doc's first
    real token row: duplicates never change the per-doc max, so no mask
    bias is needed on-device at all.  (Fully-padded docs -- impossible for
    this input distribution -- are patched on the host afterwards.)
  - D is cast to fp8 e3m4 (randn fits comfortably in its +-15.5 range;
    measured end-to-end rel err 2.9e-3 vs the 2e-2 gate) and pre-transposed
    to [DIM, 28160 doc-rows] so the device DMA is a plain contiguous
    stream; no on-chip transposes.  fp8 halves HBM traffic vs bf16.
  - Q is pre-transposed to [DIM, 512] bf16 (16 groups x 32 queries).

Per-core device program (raw bass engine streams, manual semaphores -- the
Tile framework's auto-semaphore teardown alone cost ~7us):
  - 8 chunk DMAs (2 query groups each, alternating sync/scalar HWDGE
    queues) stream D^T into SBUF, issued as each queue's first
    instructions.
  - Tensor stream: per group g, 4 col-tiled matmuls (tile_position=
    (0,32j)) put scores for doc pair (8g+2j, 8g+2j+1) in psum bank (g%8)
    partitions [32j, 32j+32); the last matmul bumps s_mm.
  - Vector stream: per group, one 3D reduce_max ([128, 2, 220] ->
    [128, 2]) into Mx [128, 32]; bumps s_red (also recycles the bank).
  - Sync stream DMAs Mx halves out after groups 7 and 15; the host sums
    the four 32-query partition blocks and de-interleaves docs.
"""

import numpy as np
import ml_dtypes

import concourse.bacc as bacc
import concourse.mybir as mybir
from concourse import bass_utils

F32 = mybir.dt.float32
BF16 = mybir.dt.bfloat16
FP8 = mybir.dt.float8e3

N_CORES = 8
B = 128          # query batches
QLEN = 32
DIM = 128
NWAY = 8
DLEN = 220
DOCS_PER_CORE = (B * NWAY) // N_CORES          # 128
ROWS_PER_CORE = DOCS_PER_CORE * DLEN           # 28160
GROUPS_PER_CORE = DOCS_PER_CORE // NWAY        # 16
GROW = NWAY * DLEN                             # 1760 rows per group
GCOLS = 2 * DLEN                               # 440 score cols per psum bank
CH_SIZES = [1, 1, 2, 2, 2, 2, 2, 2, 1, 1]      # groups per chunk
CH_G0 = [sum(CH_SIZES[:i]) for i in range(len(CH_SIZES))]
N_CHUNK = len(CH_SIZES)
BANK = 512                                     # psum bank stride (f32 elems)
BIG = 9999.0

_CACHE = {}


def _build_module():
    """Trace + compile the per-core bass module (same program on all cores)."""
    if "nc" in _CACHE:
        return _CACHE["nc"]

    nc = bacc.Bacc("TRN2", target_bir_lowering=False, debug=False)

    dt_dram = nc.dram_tensor("dt_in", [DIM, ROWS_PER_CORE], FP8,
                             kind="ExternalInput")
    qt_dram = nc.dram_tensor("qt_in", [DIM, GROUPS_PER_CORE * QLEN], BF16,
                             kind="ExternalInput")
    out_dram = nc.dram_tensor("outp", [128, 32], F32, kind="ExternalOutput")

    from contextlib import ExitStack
    with ExitStack() as stack, (
        nc.semaphore("q_dma")) as q_dma:
      ch_sems = [stack.enter_context(nc.semaphore(f"c_dma{i}"))
                 for i in range(N_CHUNK)]
      with (
        nc.semaphore("s_mm") as s_mm,      # groups of matmuls retired
        nc.semaphore("s_red") as s_red,    # reduces retired (bank recycle)
        nc.semaphore("s_out") as s_out,    # output DMA completions
        nc.sbuf_tensor("qt_sb", [128, GROUPS_PER_CORE * QLEN], BF16) as qt_sb,
        nc.sbuf_tensor("dt_sb", [128, ROWS_PER_CORE], FP8) as dt_sb,
        nc.sbuf_tensor("mx_sb", [128, 32], F32) as mx_sb,
        nc.psum_tensor("ps", [128, 8 * BANK], F32) as ps,
        nc.Block(no_gpsimd_drain=True) as block,
      ):
        @block.sync
        def _(sync):
            # warmup: tiny transfer wakes the DMA engines early
            sync.dma_start(qt_sb[0:1, :], qt_dram.ap()[0:1, :]).then_inc(
                s_out, 16)
            for ch in range(0, N_CHUNK, 2):
                a, b = CH_G0[ch] * GROW, (CH_G0[ch] + CH_SIZES[ch]) * GROW
                sync.dma_start(
                    dt_sb[:, a:b], dt_dram.ap()[:, a:b],
                ).then_inc(ch_sems[ch], 16)
            for part in range(4):
                sync.wait_ge(s_red, 4 * (part + 1))
                sync.dma_start(out_dram.ap()[:, 8 * part:8 * (part + 1)],
                               mx_sb[:, 8 * part:8 * (part + 1)]
                               ).then_inc(s_out, 16)
            sync.wait_ge(s_out, 16 * 5)

        @block.scalar
        def _(scalar):
            scalar.dma_start(qt_sb[:, :], qt_dram.ap()).then_inc(q_dma, 16)
            for ch in range(1, N_CHUNK, 2):
                a, b = CH_G0[ch] * GROW, (CH_G0[ch] + CH_SIZES[ch]) * GROW
                scalar.dma_start(
                    dt_sb[:, a:b], dt_dram.ap()[:, a:b],
                ).then_inc(ch_sems[ch], 16)

        @block.tensor
        def _(tensor):
            tensor.wait_ge(q_dma, 16)
            g2ch = {}
            for ch, g0 in enumerate(CH_G0):
                for k in range(CH_SIZES[ch]):
                    g2ch[g0 + k] = ch
            for g in range(GROUPS_PER_CORE):
                ch = g2ch[g]
                if g == CH_G0[ch]:
                    tensor.wait_ge(ch_sems[ch], 16)
                if g >= 8:
                    tensor.wait_ge(s_red, g - 7)   # psum bank recycle
                b0 = (g % 8) * BANK
                c0 = g * GROW
                for j in range(4):
                    mm = tensor.matmul(
                        ps[32 * j:32 * (j + 1), b0:b0 + GCOLS],
                        lhsT=qt_sb[:, QLEN * g:QLEN * (g + 1)],
                        rhs=dt_sb[:, c0 + GCOLS * j:c0 + GCOLS * (j + 1)],
                        start=True, stop=True,
                        tile_position=(0, 32 * j),
                        skip_group_check=True,
                    )
                mm.then_inc(s_mm, 1)

        @block.vector
        def _(vector):
            for g in range(GROUPS_PER_CORE):
                vector.wait_ge(s_mm, g + 1)
                b0 = (g % 8) * BANK
                vector.tensor_reduce(
                    mx_sb[:, 2 * g:2 * g + 2],
                    ps[:, b0:b0 + GCOLS].rearrange("p (t k) -> p t k", t=2),
                    axis=mybir.AxisListType.X,
                    op=mybir.AluOpType.max,
                ).then_inc(s_red, 1)

    nc.compile()
    _CACHE["nc"] = nc
    return nc


def _in_maps(Q, D, D_mask):
    """Host-side prep: per-core input dicts (pad-fill + cast + transpose)."""
    mask = D_mask > 0
    first_real = np.argmax(mask, axis=1)                  # [1024]
    kk = np.arange(DLEN)[None, :]
    idx = np.where(mask, kk, first_real[:, None])         # [1024, 220]
    d_filled = np.take_along_axis(D, idx[:, :, None], axis=1)
    dt_all = np.ascontiguousarray(
        d_filled.reshape(N_CORES, ROWS_PER_CORE, DIM).transpose(0, 2, 1)
    ).astype(ml_dtypes.float8_e3m4)
    qt_all = np.ascontiguousarray(
        Q.reshape(N_CORES, GROUPS_PER_CORE * QLEN, DIM).transpose(0, 2, 1)
    ).astype(ml_dtypes.bfloat16)
    return [{"dt_in": dt_all[c], "qt_in": qt_all[c]} for c in range(N_CORES)]


def kernel(Q, D, D_mask, nway):
    assert int(nway) == NWAY
    Q = np.ascontiguousarray(np.asarray(Q, dtype=np.float32))
    D = np.ascontiguousarray(np.asarray(D, dtype=np.float32))
    D_mask = np.asarray(D_mask, dtype=np.int32)

    nc = _build_module()
    res = bass_utils.run_bass_kernel_spmd(nc, _in_maps(Q, D, D_mask),
                                          core_ids=list(range(N_CORES)))

    # outp[32j+q, 2g+t] = maxsim for doc (8g+2j+t), query q; sum over q.
    s = np.arange(32)
    j = np.arange(4)
    doc_idx = 8 * (s[None, :] // 2) + 2 * j[:, None] + (s[None, :] % 2)
    out = np.empty(B * NWAY, np.float32)
    for c in range(N_CORES):
        blk = res.results[c]["outp"].reshape(4, 32, 32).sum(axis=1)  # [j, s]
        per_core = np.empty(DOCS_PER_CORE, np.float32)
        per_core[doc_idx.ravel()] = blk.ravel()
        out[c * DOCS_PER_CORE:(c + 1) * DOCS_PER_CORE] = per_core

    # fully-padded docs: reference yields exactly 32 * -9999
    fully = ~(D_mask > 0).any(axis=1)
    if fully.any():
        out[fully] = np.float32(32 * -BIG)
    return out
